# revision 9
# baseline (speedup 1.0000x reference)
"""AdditiveAttention Trainium2 kernel (8 NeuronCores, data-parallel over batch).

Reference computation (B=32, T=2048, D=U=512, fp32):
    query = values[:, -1] @ W2_w + W2_b                     # [B, U]
    keys  = values @ W1_w + W1_b                            # [B, T, U]
    score = tanh(keys + query[:, None, :]) @ V_w + V_b      # [B, T, 1]
    attn  = softmax(score, axis=1)
    out   = sum(attn * values, axis=1)                      # [B, D]

Sharding: data-parallel over B (4 batches per core), weights replicated,
no collectives.  Compute in bf16 on the TensorEngine (fp32 accumulate in
PSUM); validated end-to-end rel-err ~3e-3 vs the fp32 reference.

Per-core dataflow:
  - last rows of each batch extracted up-front via a tiny xbar DMA-transpose
    -> query for all 4 batches in 16 batched matmuls
  - per batch: valuesT (D on partitions) via xbar DMA-transpose; keysT[u] =
    sum_c W1_blk(c,u).T @ valuesT(c) -> PSUM; tanh fused on ACT with
    per-partition bias (query+W1_b+W2_b); score chunk = V.T @ tanh_keysT
  - per-batch softmax pipelined under later batches: exp (+accum Z) on ACT,
    reciprocal; unnormalized e-rows assembled to [4, T] via DMA
  - tail: PE-transpose e chunks to T-on-partitions, weighted sum with the 4
    batches col-tiled across PE column groups, 1/Z folded into the output copy
V_b drops out of softmax (constant shift).
"""

from contextlib import ExitStack

import numpy as np
import ml_dtypes

import concourse.bass as bass
import concourse.tile as tile
from concourse import bacc, mybir
from concourse.bass_utils import run_bass_kernel_spmd

BF16 = ml_dtypes.bfloat16

B, T, D, U = 32, 2048, 512, 512
NCORES = 8
BSH = B // NCORES          # 4 batches per core
P = 128
DC = D // P                # 4 chunks of D
UC = U // P                # 4 chunks of U
TS = 512                   # T tile (moving free dim) for keys GEMM
TN = T // TS               # 4
TK = T // P                # 16 chunks of T for transposes / weighted sum

_GRAPH = None


def _build_graph():
    nc = bacc.Bacc("TRN2", target_bir_lowering=False, debug=False)
    bf = mybir.dt.bfloat16
    f32 = mybir.dt.float32

    vals = nc.declare_dram_parameter("vals", [BSH, T, D], bf, isOutput=False)
    w1 = nc.declare_dram_parameter("w1", [D, U], bf, isOutput=False)
    w2 = nc.declare_dram_parameter("w2", [D, U], bf, isOutput=False)
    vw = nc.declare_dram_parameter("vw", [U, 1], bf, isOutput=False)
    bsum = nc.declare_dram_parameter("bsum", [U, 1], f32, isOutput=False)
    ident = nc.declare_dram_parameter("ident", [P, P], bf, isOutput=False)
    out_ext = nc.declare_dram_parameter("out", [BSH, D], f32, isOutput=True)

    Tanh = mybir.ActivationFunctionType.Tanh
    Exp = mybir.ActivationFunctionType.Exp

    with tile.TileContext(nc) as tc, ExitStack() as ctx:
        const = ctx.enter_context(tc.tile_pool(name="const", bufs=1))
        valt_pool = ctx.enter_context(tc.tile_pool(name="valt", bufs=2))
        nat_pool = ctx.enter_context(tc.tile_pool(name="nat", bufs=BSH))
        tk_pool = ctx.enter_context(tc.tile_pool(name="tk", bufs=3))
        sm_pool = ctx.enter_context(tc.tile_pool(name="sm", bufs=1))
        kps = ctx.enter_context(tc.tile_pool(name="kps", bufs=2, space="PSUM"))
        sps = ctx.enter_context(tc.tile_pool(name="sps", bufs=2, space="PSUM"))
        aps = ctx.enter_context(tc.tile_pool(name="aps", bufs=2, space="PSUM"))
        wps = ctx.enter_context(tc.tile_pool(name="wps", bufs=1, space="PSUM"))

        # ---- tiny loads first: last rows (for query) + weights ---------
        # lastT[p, c, j]: rows 2032..2047 of batch b transposed; col 15 of
        # free dim j is t=2047.  One xbar-transpose per batch, 16 KB each.
        lastT = const.tile([P, DC, BSH, 16], bf)
        for b in range(BSH):
            for c in range(DC):
                nc.sync.dma_start(
                    lastT[:, c, b],
                    vals.ap()[b, T - 16 : T, c * P : (c + 1) * P],
                    transpose=True,
                )

        w1_sb = const.tile([P, DC, U], bf)
        nc.sync.dma_start(w1_sb[:], w1.ap().rearrange("(c p) u -> p c u", p=P))
        w2_sb = const.tile([P, DC, U], bf)
        nc.sync.dma_start(w2_sb[:], w2.ap().rearrange("(c p) u -> p c u", p=P))
        v_sb = const.tile([P, UC], bf)
        nc.sync.dma_start(v_sb[:], vw.ap().rearrange("(c p) one -> p (c one)", p=P))
        bsum_sb = const.tile([P, UC], f32)
        nc.sync.dma_start(bsum_sb[:], bsum.ap().rearrange("(c p) one -> p (c one)", p=P))
        ident_sb = const.tile([P, P], bf)
        nc.sync.dma_start(ident_sb[:], ident.ap())

        # ---- query for all batches: q[u-chunk] = sum_c W2_blk.T @ lastcol
        qb = const.tile([P, UC, BSH], f32)
        for u in range(UC):
            qp = aps.tile([P, BSH], f32, tag="aps")
            for c in range(DC):
                nc.tensor.matmul(
                    qp[:],
                    w2_sb[:, c, u * P : (u + 1) * P],
                    lastT[:, c, :, 15],
                    start=(c == 0),
                    stop=(c == DC - 1),
                )
            nc.vector.tensor_scalar_add(qb[:, u], qp[:], bsum_sb[:, u : u + 1])

        # per-batch softmax state (all at partition 0; compute engines may
        # only address partition starts 0/32/64/96)
        score_rows = [
            sm_pool.tile([1, T], f32, name=f"srow{b}", tag=f"srow{b}")
            for b in range(BSH)
        ]
        e_rows = [
            sm_pool.tile([1, T], bf, name=f"erow{b}", tag=f"erow{b}")
            for b in range(BSH)
        ]
        zr = [
            sm_pool.tile([1, 2], f32, name=f"zr{b}", tag=f"zr{b}")
            for b in range(BSH)
        ]
        e4 = sm_pool.tile([BSH, T], bf)
        nats = []

        for b in range(BSH):
            # transposed layout: valt[p, c, t] = vals[b, t, c*128+p]
            valt = valt_pool.tile([P, DC, T], bf, tag="valt")
            for c in range(DC):
                nc.sync.dma_start(
                    valt[:, c], vals.ap()[b, :, c * P : (c + 1) * P], transpose=True
                )
            # natural layout for the weighted sum, via SWDGE (gpsimd) so the
            # bulk loads never queue ahead of transposes on the sync ring
            nat_b = nat_pool.tile([P, TK, D], bf, tag="nat")
            nc.gpsimd.dma_start(
                nat_b[:], vals.ap()[b].rearrange("(n p) d -> p n d", p=P)
            )
            nats.append(nat_b)

            # keys -> tanh -> score
            for s in range(TN):
                sp = sps.tile([1, TS], f32, tag="sps")
                for u in range(UC):
                    kp = kps.tile([P, TS], f32, tag="kps")
                    for c in range(DC):
                        nc.tensor.matmul(
                            kp[:],
                            w1_sb[:, c, u * P : (u + 1) * P],
                            valt[:, c, s * TS : (s + 1) * TS],
                            start=(c == 0),
                            stop=(c == DC - 1),
                        )
                    tkt = tk_pool.tile([P, TS], bf, tag="tk")
                    nc.scalar.activation(
                        tkt[:], kp[:], Tanh, bias=qb[:, u, b : b + 1]
                    )
                    nc.tensor.matmul(
                        sp[:],
                        v_sb[:, u : u + 1],
                        tkt[:],
                        start=(u == 0),
                        stop=(u == UC - 1),
                    )
                nc.vector.tensor_copy(
                    score_rows[b][0:1, s * TS : (s + 1) * TS], sp[:]
                )

            # per-batch softmax: exp + sum (overlaps later batches' GEMMs);
            # e stays unnormalized -- 1/Z is folded into the output copy
            nc.scalar.activation(
                e_rows[b][:], score_rows[b][:], Exp, accum_out=zr[b][:, 0:1]
            )
            nc.vector.reciprocal(zr[b][:, 1:2], zr[b][:, 0:1])
            nc.sync.dma_start(e4[b : b + 1, :], e_rows[b][:])

        # ---- tail: transpose e chunks + col-tiled weighted sum ----------
        # wsum for the 4 batches runs concurrently in 4 PE column groups
        wp = wps.tile([P, D], f32)
        at_sb = sm_pool.tile([P, TK, BSH], bf)
        for k in range(TK):
            ap_t = aps.tile([P, BSH], bf, tag="aps")
            nc.tensor.transpose(
                ap_t[:], e4[:, k * P : (k + 1) * P], ident_sb[0:BSH, 0:BSH]
            )
            nc.vector.tensor_copy(at_sb[:, k, :], ap_t[:])
            for b in range(BSH):
                nc.tensor.matmul(
                    wp[32 * b : 32 * b + 1, :],
                    at_sb[:, k, b : b + 1],
                    nats[b][:, k],
                    start=(k == 0),
                    stop=(k == TK - 1),
                    tile_position=(0, 32 * b),
                    skip_group_check=True,
                )
        for b in range(BSH):
            ob = sm_pool.tile([1, D], f32, name=f"ob{b}", tag=f"ob{b}")
            nc.vector.tensor_scalar_mul(
                ob[:], wp[32 * b : 32 * b + 1, :], zr[b][:, 1:2]
            )
            nc.sync.dma_start(out_ext.ap()[b : b + 1, :], ob[:])

    nc.finalize()
    return nc


def _get_graph():
    global _GRAPH
    if _GRAPH is None:
        _GRAPH = _build_graph()
    return _GRAPH


def _make_in_maps(values, W1_w, W1_b, W2_w, W2_b, V_w, V_b):
    vals_bf = np.ascontiguousarray(values).astype(BF16)
    w1_bf = np.ascontiguousarray(W1_w).astype(BF16)
    w2_bf = np.ascontiguousarray(W2_w).astype(BF16)
    v_bf = np.ascontiguousarray(V_w).astype(BF16)
    bsum = (
        np.asarray(W1_b, np.float32) + np.asarray(W2_b, np.float32)
    ).reshape(U, 1)
    ident = np.eye(P, dtype=BF16)

    in_maps = []
    for core in range(NCORES):
        sl = slice(core * BSH, (core + 1) * BSH)
        in_maps.append(
            {
                "vals": vals_bf[sl],
                "w1": w1_bf,
                "w2": w2_bf,
                "vw": v_bf,
                "bsum": bsum,
                "ident": ident,
            }
        )
    return in_maps


def run(inputs, trace=False, **kw):
    """Build + run on 8 cores; returns (full_output, BassKernelResults)."""
    nc = _get_graph()
    in_maps = _make_in_maps(**inputs)
    res = run_bass_kernel_spmd(
        nc, in_maps, core_ids=list(range(NCORES)), trace=trace, **kw
    )
    out = np.concatenate([np.asarray(r["out"]) for r in res.results], axis=0)
    return out.astype(np.float32), res


def kernel(**inputs) -> np.ndarray:
    out, _ = run(inputs)
    return out


# revision 10
# speedup vs baseline: 1.1722x; 1.1722x over previous
"""AdditiveAttention Trainium2 kernel (8 NeuronCores, data-parallel over batch).

Reference computation (B=32, T=2048, D=U=512, fp32):
    query = values[:, -1] @ W2_w + W2_b                     # [B, U]
    keys  = values @ W1_w + W1_b                            # [B, T, U]
    score = tanh(keys + query[:, None, :]) @ V_w + V_b      # [B, T, 1]
    attn  = softmax(score, axis=1)
    out   = sum(attn * values, axis=1)                      # [B, D]

Sharding: data-parallel over B (4 batches per core), weights replicated,
no collectives.  Compute in bf16 on the TensorEngine (fp32 accumulate in
PSUM); validated end-to-end rel-err ~3e-3 vs the fp32 reference.

Layout/scheduling notes (from perfetto traces):
  - the xbar serializes on every DMA transpose<->copy mode transition, so
    ALL 16 values-transpose DMAs run back-to-back up-front (sync queue);
    copies (weights via gpsimd before, nat/e4/out after) never interleave
  - last rows for the query come from one natural DMA + PE transposes
  - keysT accumulates into a 2-bank PSUM tile ([128, 1024], two 512-chunk
    halves) so one tanh serves two T-chunks (halves ACT op count)
  - the 4 score matmuls per u-chunk are col-tiled (tile_position) across PE
    column groups -> concurrent, out strips at partitions 0/32/64/96
  - exp reads the score strips straight from PSUM; Z and 1/Z on DVE;
    e stays unnormalized, 1/Z folds into the output copy
  - weighted sum col-tiles the 4 batches across PE column groups
V_b drops out of softmax (constant shift).
"""

from contextlib import ExitStack

import numpy as np
import ml_dtypes

import concourse.bass as bass
import concourse.tile as tile
from concourse import bacc, mybir
from concourse.bass_utils import run_bass_kernel_spmd

BF16 = ml_dtypes.bfloat16

B, T, D, U = 32, 2048, 512, 512
NCORES = 8
BSH = B // NCORES          # 4 batches per core
P = 128
DC = D // P                # 4 chunks of D
UC = U // P                # 4 chunks of U
TS = 512                   # T tile (score chunk)
TN = T // TS               # 4
SP2 = 2 * TS               # paired T tile for keys/tanh (2 PSUM banks)
NPAIR = T // SP2           # 2
TK = T // P                # 16 chunks of T for transposes / weighted sum

_GRAPH = None


def _build_graph():
    nc = bacc.Bacc("TRN2", target_bir_lowering=False, debug=False)
    bf = mybir.dt.bfloat16
    f32 = mybir.dt.float32

    vals = nc.declare_dram_parameter("vals", [BSH, T, D], bf, isOutput=False)
    w1 = nc.declare_dram_parameter("w1", [D, U], bf, isOutput=False)
    w2 = nc.declare_dram_parameter("w2", [D, U], bf, isOutput=False)
    vw = nc.declare_dram_parameter("vw", [U, 1], bf, isOutput=False)
    bsum = nc.declare_dram_parameter("bsum", [U, 1], f32, isOutput=False)
    ident = nc.declare_dram_parameter("ident", [P, P], bf, isOutput=False)
    out_ext = nc.declare_dram_parameter("out", [BSH, D], f32, isOutput=True)

    Tanh = mybir.ActivationFunctionType.Tanh
    Exp = mybir.ActivationFunctionType.Exp

    with tile.TileContext(nc) as tc, ExitStack() as ctx:
        const = ctx.enter_context(tc.tile_pool(name="const", bufs=1))
        valt_pool = ctx.enter_context(tc.tile_pool(name="valt", bufs=BSH))
        nat_pool = ctx.enter_context(tc.tile_pool(name="nat", bufs=BSH))
        tk_pool = ctx.enter_context(tc.tile_pool(name="tk", bufs=3))
        sm_pool = ctx.enter_context(tc.tile_pool(name="sm", bufs=1))
        kps = ctx.enter_context(tc.tile_pool(name="kps", bufs=2, space="PSUM"))
        sps = ctx.enter_context(tc.tile_pool(name="sps", bufs=2, space="PSUM"))
        aps = ctx.enter_context(tc.tile_pool(name="aps", bufs=2, space="PSUM"))

        # ---- constants via gpsimd (SWDGE): copies stay off the sync queue
        # and run before the xbar switches into transpose mode
        lastrows = const.tile([BSH, D], bf)
        nc.gpsimd.dma_start(lastrows[:], vals.ap()[:, T - 1, :])
        w1_sb = const.tile([P, DC, U], bf)
        nc.gpsimd.dma_start(w1_sb[:], w1.ap().rearrange("(c p) u -> p c u", p=P))
        w2_sb = const.tile([P, DC, U], bf)
        nc.gpsimd.dma_start(w2_sb[:], w2.ap().rearrange("(c p) u -> p c u", p=P))
        v_sb = const.tile([P, UC], bf)
        nc.gpsimd.dma_start(v_sb[:], vw.ap().rearrange("(c p) one -> p (c one)", p=P))
        bsum_sb = const.tile([P, UC], f32)
        nc.gpsimd.dma_start(
            bsum_sb[:], bsum.ap().rearrange("(c p) one -> p (c one)", p=P)
        )
        ident_sb = const.tile([P, P], bf)
        nc.gpsimd.dma_start(ident_sb[:], ident.ap())

        # ---- ALL values-transpose DMAs back-to-back (one xbar mode run)
        valts = []
        for b in range(BSH):
            valt = valt_pool.tile([P, DC, T], bf, tag="valt")
            for c in range(DC):
                nc.sync.dma_start(
                    valt[:, c], vals.ap()[b, :, c * P : (c + 1) * P], transpose=True
                )
            valts.append(valt)

        # natural-layout loads for the weighted sum (gpsimd; ordered after
        # the transposes by the xbar-mode dependency, needed only at tail)
        nats = []
        for b in range(BSH):
            nat_b = nat_pool.tile([P, TK, D], bf, tag="nat")
            nc.gpsimd.dma_start(
                nat_b[:], vals.ap()[b].rearrange("(n p) d -> p n d", p=P)
            )
            nats.append(nat_b)

        # ---- last rows -> lastT via PE transpose; query for all batches
        lastT = const.tile([P, DC, BSH], bf)
        for c in range(DC):
            lp = aps.tile([P, BSH], bf, tag="aps")
            nc.tensor.transpose(
                lp[:], lastrows[:, c * P : (c + 1) * P], ident_sb[0:BSH, 0:BSH]
            )
            nc.vector.tensor_copy(lastT[:, c, :], lp[:])

        qb = const.tile([P, UC, BSH], f32)
        for u in range(UC):
            qp = aps.tile([P, BSH], f32, tag="aps")
            for c in range(DC):
                nc.tensor.matmul(
                    qp[:],
                    w2_sb[:, c, u * P : (u + 1) * P],
                    lastT[:, c, :],
                    start=(c == 0),
                    stop=(c == DC - 1),
                )
            nc.vector.tensor_scalar_add(qb[:, u], qp[:], bsum_sb[:, u : u + 1])

        # per-batch softmax state (partition 0; col 0 = Z, col 1 = 1/Z)
        e_rows = [
            sm_pool.tile([1, T], bf, name=f"erow{b}", tag=f"erow{b % 2}")
            for b in range(BSH)
        ]
        zr = [
            sm_pool.tile([1, 2], f32, name=f"zr{b}", tag=f"zr{b}")
            for b in range(BSH)
        ]
        e4 = sm_pool.tile([BSH, T], bf)

        # ---- main phase: keys -> tanh -> score, batch-major ------------
        for b in range(BSH):
            valt = valts[b]
            # score strips: chunk s lives at partition 32*s of one PSUM tile
            scp = sps.tile([P, TS], f32, tag="sps")
            for u in range(UC):
                tkts = []
                for pair in range(NPAIR):
                    kp = kps.tile([P, SP2], f32, tag="kps")
                    for half in range(2):
                        s0 = pair * SP2 + half * TS
                        for c in range(DC):
                            nc.tensor.matmul(
                                kp[:, half * TS : (half + 1) * TS],
                                w1_sb[:, c, u * P : (u + 1) * P],
                                valt[:, c, s0 : s0 + TS],
                                start=(c == 0),
                                stop=(c == DC - 1),
                            )
                    tkt = tk_pool.tile([P, SP2], bf, tag="tk")
                    nc.scalar.activation(
                        tkt[:], kp[:], Tanh, bias=qb[:, u, b : b + 1]
                    )
                    tkts.append(tkt)
                # 4 score matmuls col-tiled across PE column groups
                for s in range(TN):
                    nc.tensor.matmul(
                        scp[32 * s : 32 * s + 1, :],
                        v_sb[:, u : u + 1],
                        tkts[s // 2][:, (s % 2) * TS : (s % 2 + 1) * TS],
                        start=(u == 0),
                        stop=(u == UC - 1),
                        tile_position=(0, 32 * s),
                        skip_group_check=True,
                    )
            # exp straight from the PSUM strips; Z + 1/Z on DVE
            for s in range(TN):
                nc.scalar.activation(
                    e_rows[b][0:1, s * TS : (s + 1) * TS],
                    scp[32 * s : 32 * s + 1, :],
                    Exp,
                )
            nc.vector.tensor_reduce(
                zr[b][:, 0:1], e_rows[b][:], mybir.AxisListType.X,
                mybir.AluOpType.add,
            )
            nc.vector.reciprocal(zr[b][:, 1:2], zr[b][:, 0:1])
            nc.sync.dma_start(e4[b : b + 1, :], e_rows[b][:])

        # ---- tail: transpose e chunks + col-tiled weighted sum ----------
        wp = sps.tile([P, D], f32, tag="sps")
        at_sb = sm_pool.tile([P, TK, BSH], bf)
        for k in range(TK):
            ap_t = aps.tile([P, BSH], bf, tag="aps")
            nc.tensor.transpose(
                ap_t[:], e4[:, k * P : (k + 1) * P], ident_sb[0:BSH, 0:BSH]
            )
            nc.vector.tensor_copy(at_sb[:, k, :], ap_t[:])
            for b in range(BSH):
                nc.tensor.matmul(
                    wp[32 * b : 32 * b + 1, :],
                    at_sb[:, k, b : b + 1],
                    nats[b][:, k],
                    start=(k == 0),
                    stop=(k == TK - 1),
                    tile_position=(0, 32 * b),
                    skip_group_check=True,
                )
        for b in range(BSH):
            ob = sm_pool.tile([1, D], f32, name=f"ob{b}", tag=f"ob{b}")
            nc.vector.tensor_scalar_mul(
                ob[:], wp[32 * b : 32 * b + 1, :], zr[b][:, 1:2]
            )
            nc.sync.dma_start(out_ext.ap()[b : b + 1, :], ob[:])

    nc.finalize()
    return nc


def _get_graph():
    global _GRAPH
    if _GRAPH is None:
        _GRAPH = _build_graph()
    return _GRAPH


def _make_in_maps(values, W1_w, W1_b, W2_w, W2_b, V_w, V_b):
    vals_bf = np.ascontiguousarray(values).astype(BF16)
    w1_bf = np.ascontiguousarray(W1_w).astype(BF16)
    w2_bf = np.ascontiguousarray(W2_w).astype(BF16)
    v_bf = np.ascontiguousarray(V_w).astype(BF16)
    bsum = (
        np.asarray(W1_b, np.float32) + np.asarray(W2_b, np.float32)
    ).reshape(U, 1)
    ident = np.eye(P, dtype=BF16)

    in_maps = []
    for core in range(NCORES):
        sl = slice(core * BSH, (core + 1) * BSH)
        in_maps.append(
            {
                "vals": vals_bf[sl],
                "w1": w1_bf,
                "w2": w2_bf,
                "vw": v_bf,
                "bsum": bsum,
                "ident": ident,
            }
        )
    return in_maps


def run(inputs, trace=False, **kw):
    """Build + run on 8 cores; returns (full_output, BassKernelResults)."""
    nc = _get_graph()
    in_maps = _make_in_maps(**inputs)
    res = run_bass_kernel_spmd(
        nc, in_maps, core_ids=list(range(NCORES)), trace=trace, **kw
    )
    out = np.concatenate([np.asarray(r["out"]) for r in res.results], axis=0)
    return out.astype(np.float32), res


def kernel(**inputs) -> np.ndarray:
    out, _ = run(inputs)
    return out


# revision 12
# speedup vs baseline: 1.5994x; 1.3645x over previous
"""AdditiveAttention Trainium2 kernel (8 NeuronCores, data-parallel over batch).

Reference computation (B=32, T=2048, D=U=512, fp32):
    query = values[:, -1] @ W2_w + W2_b                     # [B, U]
    keys  = values @ W1_w + W1_b                            # [B, T, U]
    score = tanh(keys + query[:, None, :]) @ V_w + V_b      # [B, T, 1]
    attn  = softmax(score, axis=1)
    out   = sum(attn * values, axis=1)                      # [B, D]

Sharding: data-parallel over B (4 batches per core), weights replicated,
no collectives.  Compute in bf16 on the TensorEngine (fp32 accumulate in
PSUM); validated end-to-end rel-err ~3e-3 vs the fp32 reference.

Layout/scheduling notes (from perfetto traces):
  - the xbar serializes on every DMA transpose<->copy mode transition, so
    ALL 16 values-transpose DMAs run back-to-back up-front (sync queue);
    copies (weights via gpsimd before, nat/e4/out after) never interleave
  - last rows for the query come from one natural DMA + PE transposes
  - keysT accumulates into a 2-bank PSUM tile ([128, 1024], two 512-chunk
    halves) so one tanh serves two T-chunks (halves ACT op count)
  - the 4 score matmuls per u-chunk are col-tiled (tile_position) across PE
    column groups -> concurrent, out strips at partitions 0/32/64/96
  - exp reads the score strips straight from PSUM; Z and 1/Z on DVE;
    e stays unnormalized, 1/Z folds into the output copy
  - weighted sum col-tiles the 4 batches across PE column groups
V_b drops out of softmax (constant shift).
"""

from contextlib import ExitStack

import numpy as np
import ml_dtypes

import concourse.bass as bass
import concourse.tile as tile
from concourse import bacc, mybir
from concourse.bass_utils import run_bass_kernel_spmd

BF16 = ml_dtypes.bfloat16

B, T, D, U = 32, 2048, 512, 512
NCORES = 8
BSH = B // NCORES          # 4 batches per core
P = 128
DC = D // P                # 4 chunks of D
UC = U // P                # 4 chunks of U
TS = 512                   # T tile (score chunk)
TN = T // TS               # 4
SP2 = 2 * TS               # paired T tile for keys/tanh (2 PSUM banks)
NPAIR = T // SP2           # 2
TK = T // P                # 16 chunks of T for transposes / weighted sum

_GRAPH = None


def _build_graph():
    nc = bacc.Bacc("TRN2", target_bir_lowering=False, debug=False)
    bf = mybir.dt.bfloat16
    f32 = mybir.dt.float32

    vals = nc.declare_dram_parameter("vals", [BSH, T, D], bf, isOutput=False)
    w1 = nc.declare_dram_parameter("w1", [D, U], bf, isOutput=False)
    w2 = nc.declare_dram_parameter("w2", [D, U], bf, isOutput=False)
    vw = nc.declare_dram_parameter("vw", [U, 1], bf, isOutput=False)
    bsum = nc.declare_dram_parameter("bsum", [U, 1], f32, isOutput=False)
    ident = nc.declare_dram_parameter("ident", [P, P], bf, isOutput=False)
    out_ext = nc.declare_dram_parameter("out", [BSH, D], f32, isOutput=True)

    Tanh = mybir.ActivationFunctionType.Tanh
    Exp = mybir.ActivationFunctionType.Exp

    with tile.TileContext(nc) as tc, ExitStack() as ctx:
        const = ctx.enter_context(tc.tile_pool(name="const", bufs=1))
        valt_pool = ctx.enter_context(tc.tile_pool(name="valt", bufs=BSH))
        nat_pool = ctx.enter_context(tc.tile_pool(name="nat", bufs=BSH))
        tk_pool = ctx.enter_context(tc.tile_pool(name="tk", bufs=3))
        sm_pool = ctx.enter_context(tc.tile_pool(name="sm", bufs=1))
        kps = ctx.enter_context(tc.tile_pool(name="kps", bufs=2, space="PSUM"))
        sps = ctx.enter_context(tc.tile_pool(name="sps", bufs=2, space="PSUM"))
        aps = ctx.enter_context(tc.tile_pool(name="aps", bufs=2, space="PSUM"))

        # ---- everything on ONE queue (sync) in strict copy -> transpose ->
        # copy order: the xbar serializes on every transpose<->copy mode
        # transition, so the stream must not alternate
        lastrows = const.tile([BSH, D], bf)
        nc.sync.dma_start(lastrows[:], vals.ap()[:, T - 1, :])
        w2_sb = const.tile([P, DC, U], bf)
        nc.sync.dma_start(w2_sb[:], w2.ap().rearrange("(c p) u -> p c u", p=P))
        bsum_sb = const.tile([P, UC], f32)
        nc.sync.dma_start(
            bsum_sb[:], bsum.ap().rearrange("(c p) one -> p (c one)", p=P)
        )
        v_sb = const.tile([P, UC], bf)
        nc.sync.dma_start(v_sb[:], vw.ap().rearrange("(c p) one -> p (c one)", p=P))
        ident_sb = const.tile([P, P], bf)
        nc.sync.dma_start(ident_sb[:], ident.ap())
        w1_sb = const.tile([P, DC, U], bf)
        nc.sync.dma_start(w1_sb[:], w1.ap().rearrange("(c p) u -> p c u", p=P))

        # ---- ALL values-transpose DMAs back-to-back (one xbar mode run)
        valts = []
        for b in range(BSH):
            valt = valt_pool.tile([P, DC, T], bf, tag="valt")
            for c in range(DC):
                nc.sync.dma_start(
                    valt[:, c], vals.ap()[b, :, c * P : (c + 1) * P], transpose=True
                )
            valts.append(valt)

        # natural-layout loads for the weighted sum (sync FIFO puts them
        # after all transposes; needed only at the tail)
        nats = []
        for b in range(BSH):
            nat_b = nat_pool.tile([P, TK, D], bf, tag="nat")
            nc.sync.dma_start(
                nat_b[:], vals.ap()[b].rearrange("(n p) d -> p n d", p=P)
            )
            nats.append(nat_b)

        # ---- last rows -> lastT via PE transpose; query for all batches
        lastT = const.tile([P, DC, BSH], bf)
        for c in range(DC):
            lp = aps.tile([P, BSH], bf, tag="aps")
            nc.tensor.transpose(
                lp[:], lastrows[:, c * P : (c + 1) * P], ident_sb[0:BSH, 0:BSH]
            )
            nc.vector.tensor_copy(lastT[:, c, :], lp[:])

        qb = const.tile([P, UC, BSH], f32)
        for u in range(UC):
            qp = aps.tile([P, BSH], f32, tag="aps")
            for c in range(DC):
                nc.tensor.matmul(
                    qp[:],
                    w2_sb[:, c, u * P : (u + 1) * P],
                    lastT[:, c, :],
                    start=(c == 0),
                    stop=(c == DC - 1),
                )
            nc.vector.tensor_scalar_add(qb[:, u], qp[:], bsum_sb[:, u : u + 1])

        # per-batch softmax state (partition 0; col 0 = Z, col 1 = 1/Z)
        e_rows = [
            sm_pool.tile([1, T], bf, name=f"erow{b}", tag=f"erow{b % 2}")
            for b in range(BSH)
        ]
        zr = [
            sm_pool.tile([1, 2], f32, name=f"zr{b}", tag=f"zr{b}")
            for b in range(BSH)
        ]
        zparts = [
            sm_pool.tile([1, TN], f32, name=f"zp{b}", tag=f"zp{b}")
            for b in range(BSH)
        ]
        e4 = sm_pool.tile([BSH, T], bf)

        # ---- main phase: keys -> tanh -> score, batch-major ------------
        for b in range(BSH):
            valt = valts[b]
            # score strips: chunk s lives at partition 32*s of one PSUM tile
            scp = sps.tile([P, TS], f32, tag="sps")
            for u in range(UC):
                tkts = []
                for pair in range(NPAIR):
                    kp = kps.tile([P, SP2], f32, tag="kps")
                    for half in range(2):
                        s0 = pair * SP2 + half * TS
                        for c in range(DC):
                            nc.tensor.matmul(
                                kp[:, half * TS : (half + 1) * TS],
                                w1_sb[:, c, u * P : (u + 1) * P],
                                valt[:, c, s0 : s0 + TS],
                                start=(c == 0),
                                stop=(c == DC - 1),
                            )
                    tkt = tk_pool.tile([P, SP2], bf, tag="tk")
                    nc.scalar.activation(
                        tkt[:], kp[:], Tanh, bias=qb[:, u, b : b + 1]
                    )
                    tkts.append(tkt)
                # 4 score matmuls col-tiled across PE column groups
                for s in range(TN):
                    nc.tensor.matmul(
                        scp[32 * s : 32 * s + 1, :],
                        v_sb[:, u : u + 1],
                        tkts[s // 2][:, (s % 2) * TS : (s % 2 + 1) * TS],
                        start=(u == 0),
                        stop=(u == UC - 1),
                        tile_position=(0, 32 * s),
                        skip_group_check=True,
                    )
            # exp straight from the PSUM strips; Z + 1/Z on DVE
            for s in range(TN):
                nc.scalar.activation(
                    e_rows[b][0:1, s * TS : (s + 1) * TS],
                    scp[32 * s : 32 * s + 1, :],
                    Exp,
                )
            nc.vector.tensor_reduce(
                zr[b][:, 0:1], e_rows[b][:], mybir.AxisListType.X,
                mybir.AluOpType.add,
            )
            nc.vector.reciprocal(zr[b][:, 1:2], zr[b][:, 0:1])
            nc.sync.dma_start(e4[b : b + 1, :], e_rows[b][:])

        # ---- tail: transpose e chunks + col-tiled weighted sum ----------
        wp = sps.tile([P, D], f32, tag="sps")
        at_sb = sm_pool.tile([P, TK, BSH], bf)
        for k in range(TK):
            ap_t = aps.tile([P, BSH], bf, tag="aps")
            nc.tensor.transpose(
                ap_t[:], e4[:, k * P : (k + 1) * P], ident_sb[0:BSH, 0:BSH]
            )
            nc.vector.tensor_copy(at_sb[:, k, :], ap_t[:])
            for b in range(BSH):
                nc.tensor.matmul(
                    wp[32 * b : 32 * b + 1, :],
                    at_sb[:, k, b : b + 1],
                    nats[b][:, k],
                    start=(k == 0),
                    stop=(k == TK - 1),
                    tile_position=(0, 32 * b),
                    skip_group_check=True,
                )
        for b in range(BSH):
            ob = sm_pool.tile([1, D], f32, name=f"ob{b}", tag=f"ob{b}")
            nc.vector.tensor_scalar_mul(
                ob[:], wp[32 * b : 32 * b + 1, :], zr[b][:, 1:2]
            )
            nc.sync.dma_start(out_ext.ap()[b : b + 1, :], ob[:])

    nc.finalize()
    return nc


def _get_graph():
    global _GRAPH
    if _GRAPH is None:
        _GRAPH = _build_graph()
    return _GRAPH


def _make_in_maps(values, W1_w, W1_b, W2_w, W2_b, V_w, V_b):
    vals_bf = np.ascontiguousarray(values).astype(BF16)
    w1_bf = np.ascontiguousarray(W1_w).astype(BF16)
    w2_bf = np.ascontiguousarray(W2_w).astype(BF16)
    v_bf = np.ascontiguousarray(V_w).astype(BF16)
    bsum = (
        np.asarray(W1_b, np.float32) + np.asarray(W2_b, np.float32)
    ).reshape(U, 1)
    ident = np.eye(P, dtype=BF16)

    in_maps = []
    for core in range(NCORES):
        sl = slice(core * BSH, (core + 1) * BSH)
        in_maps.append(
            {
                "vals": vals_bf[sl],
                "w1": w1_bf,
                "w2": w2_bf,
                "vw": v_bf,
                "bsum": bsum,
                "ident": ident,
            }
        )
    return in_maps


def run(inputs, trace=False, **kw):
    """Build + run on 8 cores; returns (full_output, BassKernelResults)."""
    nc = _get_graph()
    in_maps = _make_in_maps(**inputs)
    res = run_bass_kernel_spmd(
        nc, in_maps, core_ids=list(range(NCORES)), trace=trace, **kw
    )
    out = np.concatenate([np.asarray(r["out"]) for r in res.results], axis=0)
    return out.astype(np.float32), res


def kernel(**inputs) -> np.ndarray:
    out, _ = run(inputs)
    return out


# revision 15
# speedup vs baseline: 1.7263x; 1.0793x over previous
"""AdditiveAttention Trainium2 kernel (8 NeuronCores, data-parallel over batch).

Reference computation (B=32, T=2048, D=U=512, fp32):
    query = values[:, -1] @ W2_w + W2_b                     # [B, U]
    keys  = values @ W1_w + W1_b                            # [B, T, U]
    score = tanh(keys + query[:, None, :]) @ V_w + V_b      # [B, T, 1]
    attn  = softmax(score, axis=1)
    out   = sum(attn * values, axis=1)                      # [B, D]

Sharding: data-parallel over B (4 batches per core), weights replicated,
no collectives.  Compute in bf16 on the TensorEngine (fp32 accumulate in
PSUM); validated end-to-end rel-err ~3e-3 vs the fp32 reference.

Layout/scheduling notes (from perfetto traces):
  - the xbar serializes on every DMA transpose<->copy mode transition, so
    ALL 16 values-transpose DMAs run back-to-back up-front (sync queue);
    copies (weights via gpsimd before, nat/e4/out after) never interleave
  - last rows for the query come from one natural DMA + PE transposes
  - keysT accumulates into a 2-bank PSUM tile ([128, 1024], two 512-chunk
    halves) so one tanh serves two T-chunks (halves ACT op count)
  - the 4 score matmuls per u-chunk are col-tiled (tile_position) across PE
    column groups -> concurrent, out strips at partitions 0/32/64/96
  - exp reads the score strips straight from PSUM; Z and 1/Z on DVE;
    e stays unnormalized, 1/Z folds into the output copy
  - weighted sum col-tiles the 4 batches across PE column groups
V_b drops out of softmax (constant shift).
"""

from contextlib import ExitStack

import numpy as np
import ml_dtypes

import concourse.bass as bass
import concourse.tile as tile
from concourse import bacc, mybir
from concourse.bass_utils import run_bass_kernel_spmd

BF16 = ml_dtypes.bfloat16

B, T, D, U = 32, 2048, 512, 512
NCORES = 8
BSH = B // NCORES          # 4 batches per core
P = 128
DC = D // P                # 4 chunks of D
UC = U // P                # 4 chunks of U
TS = 512                   # T tile (score chunk)
TN = T // TS               # 4
SP2 = 2 * TS               # paired T tile for keys/tanh (2 PSUM banks)
NPAIR = T // SP2           # 2
TK = T // P                # 16 chunks of T for transposes / weighted sum

_GRAPH = None


def _build_graph():
    nc = bacc.Bacc("TRN2", target_bir_lowering=False, debug=False)
    bf = mybir.dt.bfloat16
    f32 = mybir.dt.float32

    vals = nc.declare_dram_parameter("vals", [BSH, T, D], bf, isOutput=False)
    w1 = nc.declare_dram_parameter("w1", [D, U], bf, isOutput=False)
    w2 = nc.declare_dram_parameter("w2", [D, U], bf, isOutput=False)
    vw = nc.declare_dram_parameter("vw", [U, 1], bf, isOutput=False)
    bsum = nc.declare_dram_parameter("bsum", [U, 1], f32, isOutput=False)
    ident = nc.declare_dram_parameter("ident", [BSH, BSH], bf, isOutput=False)
    out_ext = nc.declare_dram_parameter("out", [BSH, D], f32, isOutput=True)

    Tanh = mybir.ActivationFunctionType.Tanh
    Exp = mybir.ActivationFunctionType.Exp

    with tile.TileContext(nc) as tc, ExitStack() as ctx:
        const = ctx.enter_context(tc.tile_pool(name="const", bufs=1))
        valt_pool = ctx.enter_context(tc.tile_pool(name="valt", bufs=BSH))
        nat_pool = ctx.enter_context(tc.tile_pool(name="nat", bufs=BSH))
        tk_pool = ctx.enter_context(tc.tile_pool(name="tk", bufs=3))
        sm_pool = ctx.enter_context(tc.tile_pool(name="sm", bufs=1))
        kps = ctx.enter_context(tc.tile_pool(name="kps", bufs=2, space="PSUM"))
        sps = ctx.enter_context(tc.tile_pool(name="sps", bufs=2, space="PSUM"))
        aps = ctx.enter_context(tc.tile_pool(name="aps", bufs=2, space="PSUM"))

        # ---- everything on ONE queue (sync) in strict copy -> transpose ->
        # copy order: the xbar serializes on every transpose<->copy mode
        # transition, so the stream must not alternate
        lastrows = const.tile([BSH, D], bf)
        nc.sync.dma_start(lastrows[:], vals.ap()[:, T - 1, :])
        w2_sb = const.tile([P, DC, U], bf)
        nc.sync.dma_start(w2_sb[:], w2.ap().rearrange("(c p) u -> p c u", p=P))
        ident_sb = const.tile([BSH, BSH], bf)
        nc.sync.dma_start(ident_sb[:], ident.ap())
        w1_sb = const.tile([P, DC, U], bf)
        nc.sync.dma_start(w1_sb[:], w1.ap().rearrange("(c p) u -> p c u", p=P))
        bsum_sb = const.tile([P, UC], f32)
        nc.sync.dma_start(
            bsum_sb[:], bsum.ap().rearrange("(c p) one -> p (c one)", p=P)
        )
        v_sb = const.tile([P, UC], bf)
        nc.sync.dma_start(v_sb[:], vw.ap().rearrange("(c p) one -> p (c one)", p=P))

        # ---- ALL values-transpose DMAs back-to-back (one xbar mode run)
        valts = []
        for b in range(BSH):
            valt = valt_pool.tile([P, DC, T], bf, tag="valt")
            if b == 0:
                for h in range(2):
                    for c in range(DC):
                        nc.sync.dma_start(
                            valt[:, c, h * (T // 2) : (h + 1) * (T // 2)],
                            vals.ap()[
                                b, h * (T // 2) : (h + 1) * (T // 2),
                                c * P : (c + 1) * P,
                            ],
                            transpose=True,
                        )
            else:
                for c in range(DC):
                    nc.sync.dma_start(
                        valt[:, c], vals.ap()[b, :, c * P : (c + 1) * P],
                        transpose=True,
                    )
            valts.append(valt)

        # natural-layout loads for the weighted sum (sync FIFO puts them
        # after all transposes; needed only at the tail)
        nats = []
        for b in range(BSH):
            nat_b = nat_pool.tile([P, TK, D], bf, tag="nat")
            nc.sync.dma_start(
                nat_b[:], vals.ap()[b].rearrange("(n p) d -> p n d", p=P)
            )
            nats.append(nat_b)

        # ---- last rows -> lastT via PE transpose; query for all batches
        lastT = const.tile([P, DC, BSH], bf)
        for c in range(DC):
            lp = aps.tile([P, BSH], bf, tag="aps")
            nc.tensor.transpose(
                lp[:], lastrows[:, c * P : (c + 1) * P], ident_sb[:]
            )
            nc.vector.tensor_copy(lastT[:, c, :], lp[:])

        qb = const.tile([P, UC, BSH], f32)
        for u in range(UC):
            qp = aps.tile([P, BSH], f32, tag="aps")
            for c in range(DC):
                nc.tensor.matmul(
                    qp[:],
                    w2_sb[:, c, u * P : (u + 1) * P],
                    lastT[:, c, :],
                    start=(c == 0),
                    stop=(c == DC - 1),
                )
            nc.vector.tensor_scalar_add(qb[:, u], qp[:], bsum_sb[:, u : u + 1])

        # per-batch softmax state (partition 0; col 0 = Z, col 1 = 1/Z)
        e_rows = [
            sm_pool.tile([1, T], bf, name=f"erow{b}", tag=f"erow{b}")
            for b in range(BSH)
        ]
        zr = [
            sm_pool.tile([1, 2], f32, name=f"zr{b}", tag=f"zr{b}")
            for b in range(BSH)
        ]
        zparts = [
            sm_pool.tile([1, TN], f32, name=f"zp{b}", tag=f"zp{b}")
            for b in range(BSH)
        ]
        e4 = sm_pool.tile([BSH, T], bf)

        # ---- main phase: keys -> tanh -> score, batch-major ------------
        for b in range(BSH):
            valt = valts[b]
            # score strips: chunk s lives at partition 32*s of one PSUM tile
            scp = sps.tile([P, TS], f32, tag="sps")
            for u in range(UC):
                tkts = []
                for pair in range(NPAIR):
                    kp = kps.tile([P, SP2], f32, tag="kps")
                    for half in range(2):
                        s0 = pair * SP2 + half * TS
                        for c in range(DC):
                            nc.tensor.matmul(
                                kp[:, half * TS : (half + 1) * TS],
                                w1_sb[:, c, u * P : (u + 1) * P],
                                valt[:, c, s0 : s0 + TS],
                                start=(c == 0),
                                stop=(c == DC - 1),
                            )
                    tkt = tk_pool.tile([P, SP2], bf, tag="tk")
                    nc.scalar.activation(
                        tkt[:], kp[:], Tanh, bias=qb[:, u, b : b + 1]
                    )
                    tkts.append(tkt)
                # 4 score matmuls col-tiled across PE column groups
                for s in range(TN):
                    nc.tensor.matmul(
                        scp[32 * s : 32 * s + 1, :],
                        v_sb[:, u : u + 1],
                        tkts[s // 2][:, (s % 2) * TS : (s % 2 + 1) * TS],
                        start=(u == 0),
                        stop=(u == UC - 1),
                        tile_position=(0, 32 * s),
                        skip_group_check=True,
                    )
            # exp straight from the PSUM strips; Z + 1/Z on DVE
            for s in range(TN):
                nc.scalar.activation(
                    e_rows[b][0:1, s * TS : (s + 1) * TS],
                    scp[32 * s : 32 * s + 1, :],
                    Exp,
                )
            nc.vector.tensor_reduce(
                zr[b][:, 0:1], e_rows[b][:], mybir.AxisListType.X,
                mybir.AluOpType.add,
            )
            nc.vector.reciprocal(zr[b][:, 1:2], zr[b][:, 0:1])
            nc.sync.dma_start(e4[b : b + 1, :], e_rows[b][:])

        # ---- tail: transpose e chunks + col-tiled weighted sum ----------
        wp = sps.tile([P, D], f32, tag="sps")
        at_sb = sm_pool.tile([P, TK, BSH], bf)
        for k in range(TK):
            ap_t = aps.tile([P, BSH], bf, tag="aps")
            nc.tensor.transpose(
                ap_t[:], e4[:, k * P : (k + 1) * P], ident_sb[:]
            )
            nc.vector.tensor_copy(at_sb[:, k, :], ap_t[:])
            for b in range(BSH):
                nc.tensor.matmul(
                    wp[32 * b : 32 * b + 1, :],
                    at_sb[:, k, b : b + 1],
                    nats[b][:, k],
                    start=(k == 0),
                    stop=(k == TK - 1),
                    tile_position=(0, 32 * b),
                    skip_group_check=True,
                )
        for b in range(BSH):
            ob = sm_pool.tile([1, D], f32, name=f"ob{b}", tag=f"ob{b}")
            nc.vector.tensor_scalar_mul(
                ob[:], wp[32 * b : 32 * b + 1, :], zr[b][:, 1:2]
            )
            nc.sync.dma_start(out_ext.ap()[b : b + 1, :], ob[:])

    nc.finalize()
    return nc


def _get_graph():
    global _GRAPH
    if _GRAPH is None:
        _GRAPH = _build_graph()
    return _GRAPH


def _make_in_maps(values, W1_w, W1_b, W2_w, W2_b, V_w, V_b):
    vals_bf = np.ascontiguousarray(values).astype(BF16)
    w1_bf = np.ascontiguousarray(W1_w).astype(BF16)
    w2_bf = np.ascontiguousarray(W2_w).astype(BF16)
    v_bf = np.ascontiguousarray(V_w).astype(BF16)
    bsum = (
        np.asarray(W1_b, np.float32) + np.asarray(W2_b, np.float32)
    ).reshape(U, 1)
    ident = np.eye(BSH, dtype=BF16)

    in_maps = []
    for core in range(NCORES):
        sl = slice(core * BSH, (core + 1) * BSH)
        in_maps.append(
            {
                "vals": vals_bf[sl],
                "w1": w1_bf,
                "w2": w2_bf,
                "vw": v_bf,
                "bsum": bsum,
                "ident": ident,
            }
        )
    return in_maps


def run(inputs, trace=False, **kw):
    """Build + run on 8 cores; returns (full_output, BassKernelResults)."""
    nc = _get_graph()
    in_maps = _make_in_maps(**inputs)
    res = run_bass_kernel_spmd(
        nc, in_maps, core_ids=list(range(NCORES)), trace=trace, **kw
    )
    out = np.concatenate([np.asarray(r["out"]) for r in res.results], axis=0)
    return out.astype(np.float32), res


def kernel(**inputs) -> np.ndarray:
    out, _ = run(inputs)
    return out


# revision 16
# speedup vs baseline: 1.7301x; 1.0022x over previous
"""AdditiveAttention Trainium2 kernel (8 NeuronCores, data-parallel over batch).

Reference computation (B=32, T=2048, D=U=512, fp32):
    query = values[:, -1] @ W2_w + W2_b                     # [B, U]
    keys  = values @ W1_w + W1_b                            # [B, T, U]
    score = tanh(keys + query[:, None, :]) @ V_w + V_b      # [B, T, 1]
    attn  = softmax(score, axis=1)
    out   = sum(attn * values, axis=1)                      # [B, D]

Sharding: data-parallel over B (4 batches per core), weights replicated,
no collectives.  Compute in bf16 on the TensorEngine (fp32 accumulate in
PSUM); validated end-to-end rel-err ~3e-3 vs the fp32 reference.

Layout/scheduling notes (from perfetto traces):
  - the xbar serializes on every DMA transpose<->copy mode transition, so
    ALL 16 values-transpose DMAs run back-to-back up-front (sync queue);
    copies (weights via gpsimd before, nat/e4/out after) never interleave
  - last rows for the query come from one natural DMA + PE transposes
  - keysT accumulates into a 2-bank PSUM tile ([128, 1024], two 512-chunk
    halves) so one tanh serves two T-chunks (halves ACT op count)
  - the 4 score matmuls per u-chunk are col-tiled (tile_position) across PE
    column groups -> concurrent, out strips at partitions 0/32/64/96
  - exp reads the score strips straight from PSUM; Z and 1/Z on DVE;
    e stays unnormalized, 1/Z folds into the output copy
  - weighted sum col-tiles the 4 batches across PE column groups
V_b drops out of softmax (constant shift).
"""

from contextlib import ExitStack

import numpy as np
import ml_dtypes

import concourse.bass as bass
import concourse.tile as tile
from concourse import bacc, mybir
from concourse.bass_utils import run_bass_kernel_spmd

BF16 = ml_dtypes.bfloat16

B, T, D, U = 32, 2048, 512, 512
NCORES = 8
BSH = B // NCORES          # 4 batches per core
P = 128
DC = D // P                # 4 chunks of D
UC = U // P                # 4 chunks of U
TS = 512                   # T tile (score chunk)
TN = T // TS               # 4
SP2 = 2 * TS               # paired T tile for keys/tanh (2 PSUM banks)
NPAIR = T // SP2           # 2
TK = T // P                # 16 chunks of T for transposes / weighted sum

_GRAPH = None


def _build_graph():
    nc = bacc.Bacc("TRN2", target_bir_lowering=False, debug=False)
    bf = mybir.dt.bfloat16
    f32 = mybir.dt.float32

    vals = nc.declare_dram_parameter("vals", [BSH, T, D], bf, isOutput=False)
    w1 = nc.declare_dram_parameter("w1", [D, U], bf, isOutput=False)
    w2 = nc.declare_dram_parameter("w2", [D, U], bf, isOutput=False)
    vw = nc.declare_dram_parameter("vw", [U, 1], bf, isOutput=False)
    bsum = nc.declare_dram_parameter("bsum", [U, 1], f32, isOutput=False)
    ident = nc.declare_dram_parameter("ident", [BSH, BSH], bf, isOutput=False)
    out_ext = nc.declare_dram_parameter("out", [BSH, D], f32, isOutput=True)

    Tanh = mybir.ActivationFunctionType.Tanh
    Exp = mybir.ActivationFunctionType.Exp

    with tile.TileContext(nc) as tc, ExitStack() as ctx:
        const = ctx.enter_context(tc.tile_pool(name="const", bufs=1))
        valt_pool = ctx.enter_context(tc.tile_pool(name="valt", bufs=BSH))
        nat_pool = ctx.enter_context(tc.tile_pool(name="nat", bufs=BSH))
        tk_pool = ctx.enter_context(tc.tile_pool(name="tk", bufs=3))
        sm_pool = ctx.enter_context(tc.tile_pool(name="sm", bufs=1))
        kps = ctx.enter_context(tc.tile_pool(name="kps", bufs=2, space="PSUM"))
        sps = ctx.enter_context(tc.tile_pool(name="sps", bufs=2, space="PSUM"))
        aps = ctx.enter_context(tc.tile_pool(name="aps", bufs=2, space="PSUM"))

        # ---- everything on ONE queue (sync) in strict copy -> transpose ->
        # copy order: the xbar serializes on every transpose<->copy mode
        # transition, so the stream must not alternate
        lastrows = const.tile([BSH, D], bf)
        nc.sync.dma_start(lastrows[:], vals.ap()[:, T - 1, :])
        w2_sb = const.tile([P, DC, U], bf)
        nc.sync.dma_start(w2_sb[:], w2.ap().rearrange("(c p) u -> p c u", p=P))
        ident_sb = const.tile([BSH, BSH], bf)
        nc.sync.dma_start(ident_sb[:], ident.ap())
        w1_sb = const.tile([P, DC, U], bf)
        nc.sync.dma_start(w1_sb[:], w1.ap().rearrange("(c p) u -> p c u", p=P))
        bsum_sb = const.tile([P, UC], f32)
        nc.sync.dma_start(
            bsum_sb[:], bsum.ap().rearrange("(c p) one -> p (c one)", p=P)
        )
        v_sb = const.tile([P, UC], bf)
        nc.sync.dma_start(v_sb[:], vw.ap().rearrange("(c p) one -> p (c one)", p=P))

        # ---- ALL values-transpose DMAs back-to-back (one xbar mode run)
        valts = []
        for b in range(BSH):
            valt = valt_pool.tile([P, DC, T], bf, tag="valt")
            if b <= 1:
                for h in range(2):
                    for c in range(DC):
                        nc.sync.dma_start(
                            valt[:, c, h * (T // 2) : (h + 1) * (T // 2)],
                            vals.ap()[
                                b, h * (T // 2) : (h + 1) * (T // 2),
                                c * P : (c + 1) * P,
                            ],
                            transpose=True,
                        )
            else:
                for c in range(DC):
                    nc.sync.dma_start(
                        valt[:, c], vals.ap()[b, :, c * P : (c + 1) * P],
                        transpose=True,
                    )
            valts.append(valt)

        # natural-layout loads for the weighted sum (sync FIFO puts them
        # after all transposes; needed only at the tail)
        nats = []
        for b in range(BSH):
            nat_b = nat_pool.tile([P, TK, D], bf, tag="nat")
            nc.sync.dma_start(
                nat_b[:], vals.ap()[b].rearrange("(n p) d -> p n d", p=P)
            )
            nats.append(nat_b)

        # ---- last rows -> lastT via PE transpose; query for all batches
        lastT = const.tile([P, DC, BSH], bf)
        for c in range(DC):
            lp = aps.tile([P, BSH], bf, tag="aps")
            nc.tensor.transpose(
                lp[:], lastrows[:, c * P : (c + 1) * P], ident_sb[:]
            )
            nc.vector.tensor_copy(lastT[:, c, :], lp[:])

        qb = const.tile([P, UC, BSH], f32)
        for u in range(UC):
            qp = aps.tile([P, BSH], f32, tag="aps")
            for c in range(DC):
                nc.tensor.matmul(
                    qp[:],
                    w2_sb[:, c, u * P : (u + 1) * P],
                    lastT[:, c, :],
                    start=(c == 0),
                    stop=(c == DC - 1),
                )
            nc.vector.tensor_scalar_add(qb[:, u], qp[:], bsum_sb[:, u : u + 1])

        # per-batch softmax state (partition 0; col 0 = Z, col 1 = 1/Z)
        e_rows = [
            sm_pool.tile([1, T], bf, name=f"erow{b}", tag=f"erow{b}")
            for b in range(BSH)
        ]
        zr = [
            sm_pool.tile([1, 2], f32, name=f"zr{b}", tag=f"zr{b}")
            for b in range(BSH)
        ]
        zparts = [
            sm_pool.tile([1, TN], f32, name=f"zp{b}", tag=f"zp{b}")
            for b in range(BSH)
        ]
        e4 = sm_pool.tile([BSH, T], bf)

        # ---- main phase: keys -> tanh -> score, batch-major ------------
        for b in range(BSH):
            valt = valts[b]
            # score strips: chunk s lives at partition 32*s of one PSUM tile
            scp = sps.tile([P, TS], f32, tag="sps")
            for u in range(UC):
                tkts = []
                for pair in range(NPAIR):
                    kp = kps.tile([P, SP2], f32, tag="kps")
                    for half in range(2):
                        s0 = pair * SP2 + half * TS
                        for c in range(DC):
                            nc.tensor.matmul(
                                kp[:, half * TS : (half + 1) * TS],
                                w1_sb[:, c, u * P : (u + 1) * P],
                                valt[:, c, s0 : s0 + TS],
                                start=(c == 0),
                                stop=(c == DC - 1),
                            )
                    tkt = tk_pool.tile([P, SP2], bf, tag="tk")
                    nc.scalar.activation(
                        tkt[:], kp[:], Tanh, bias=qb[:, u, b : b + 1]
                    )
                    tkts.append(tkt)
                # 4 score matmuls col-tiled across PE column groups
                for s in range(TN):
                    nc.tensor.matmul(
                        scp[32 * s : 32 * s + 1, :],
                        v_sb[:, u : u + 1],
                        tkts[s // 2][:, (s % 2) * TS : (s % 2 + 1) * TS],
                        start=(u == 0),
                        stop=(u == UC - 1),
                        tile_position=(0, 32 * s),
                        skip_group_check=True,
                    )
            # exp straight from the PSUM strips; per-chunk Z partials and
            # per-chunk e4 assembly keep batch 3's tail chain short
            zp = zparts[b]
            for s in range(TN):
                nc.scalar.activation(
                    e_rows[b][0:1, s * TS : (s + 1) * TS],
                    scp[32 * s : 32 * s + 1, :],
                    Exp,
                )
                nc.vector.tensor_reduce(
                    zp[:, s : s + 1],
                    e_rows[b][0:1, s * TS : (s + 1) * TS],
                    mybir.AxisListType.X, mybir.AluOpType.add,
                )
                nc.sync.dma_start(
                    e4[b : b + 1, s * TS : (s + 1) * TS],
                    e_rows[b][0:1, s * TS : (s + 1) * TS],
                )
            nc.vector.tensor_reduce(
                zr[b][:, 0:1], zp[:], mybir.AxisListType.X, mybir.AluOpType.add,
            )
            nc.vector.reciprocal(zr[b][:, 1:2], zr[b][:, 0:1])

        # ---- tail: transpose e chunks + col-tiled weighted sum ----------
        wp = sps.tile([P, D], f32, tag="sps")
        at_sb = sm_pool.tile([P, TK, BSH], bf)
        for k in range(TK):
            ap_t = aps.tile([P, BSH], bf, tag="aps")
            nc.tensor.transpose(
                ap_t[:], e4[:, k * P : (k + 1) * P], ident_sb[:]
            )
            nc.vector.tensor_copy(at_sb[:, k, :], ap_t[:])
            for b in range(BSH):
                nc.tensor.matmul(
                    wp[32 * b : 32 * b + 1, :],
                    at_sb[:, k, b : b + 1],
                    nats[b][:, k],
                    start=(k == 0),
                    stop=(k == TK - 1),
                    tile_position=(0, 32 * b),
                    skip_group_check=True,
                )
        for b in range(BSH):
            ob = sm_pool.tile([1, D], f32, name=f"ob{b}", tag=f"ob{b}")
            nc.vector.tensor_scalar_mul(
                ob[:], wp[32 * b : 32 * b + 1, :], zr[b][:, 1:2]
            )
            nc.sync.dma_start(out_ext.ap()[b : b + 1, :], ob[:])

    nc.finalize()
    return nc


def _get_graph():
    global _GRAPH
    if _GRAPH is None:
        _GRAPH = _build_graph()
    return _GRAPH


def _make_in_maps(values, W1_w, W1_b, W2_w, W2_b, V_w, V_b):
    vals_bf = np.ascontiguousarray(values).astype(BF16)
    w1_bf = np.ascontiguousarray(W1_w).astype(BF16)
    w2_bf = np.ascontiguousarray(W2_w).astype(BF16)
    v_bf = np.ascontiguousarray(V_w).astype(BF16)
    bsum = (
        np.asarray(W1_b, np.float32) + np.asarray(W2_b, np.float32)
    ).reshape(U, 1)
    ident = np.eye(BSH, dtype=BF16)

    in_maps = []
    for core in range(NCORES):
        sl = slice(core * BSH, (core + 1) * BSH)
        in_maps.append(
            {
                "vals": vals_bf[sl],
                "w1": w1_bf,
                "w2": w2_bf,
                "vw": v_bf,
                "bsum": bsum,
                "ident": ident,
            }
        )
    return in_maps


def run(inputs, trace=False, **kw):
    """Build + run on 8 cores; returns (full_output, BassKernelResults)."""
    nc = _get_graph()
    in_maps = _make_in_maps(**inputs)
    res = run_bass_kernel_spmd(
        nc, in_maps, core_ids=list(range(NCORES)), trace=trace, **kw
    )
    out = np.concatenate([np.asarray(r["out"]) for r in res.results], axis=0)
    return out.astype(np.float32), res


def kernel(**inputs) -> np.ndarray:
    out, _ = run(inputs)
    return out
